# revision 2
# baseline (speedup 1.0000x reference)
"""Trainium2 Bass kernel for nn_CustomLoss_11630771438153 (retrieval_knn).

Strategy v3: shard the xnorm-sorted database row-wise across 8 cores
(12500 rows each, zero-padded to 12512 = 2*1024 + 5*2048 + 224). Each
core computes raw inner-product scores s' = tq2.T @ X_shard (bf16) on
the TensorEngine and screens EVERY score element with the two engines
that can read PSUM, balanced so both finish together:

  - DVE `tensor_reduce` max over 64-col windows on the first ~53% of
    each psum tile -> tight per-window upper/lower bounds.
  - ScalarE `activation(Exp, bias=-U_q/T, scale=1/T, accum_out=..)` on
    the rest -> log-sum-exp bounds per ~960-col group.

The two query blocks are processed per-X-tile interleaved (qb0 then
qb1) so each X tile is consumed right after its DMA. No warmup/filler
matmul spam (2 warmup MMs only), exp table load overlapped with the
DMA head, and outputs are small f32 tensors DMA'd in two overlapping
pieces.

Host: merge per-unit bounds; t_q = 16th best lower bound; rescan all
units with UB >= t_q exactly in numpy (validating the device-vs-exact
deviation budget delta, escalating on failure); compute the exact loss
from the resulting top-16 sets.
"""

import os
import sys
import time

sys.path.insert(0, "/opt/trn_rl_repo")

MERGE_DEBUG = bool(os.environ.get("MERGE_DEBUG"))

import ml_dtypes
import numpy as np

import concourse.tile as tile
from concourse import bacc, mybir
from concourse.bass_utils import run_bass_kernel_spmd

# Problem constants (hardcoded per the harness contract).
B = 256  # queries
D = 128  # feature dim
N = 100000  # database size
K = 16  # neighbors
TAU = 0.1
BETA = 1.0
LAMB = 1e-4
EPS = 1e-8

N_CORES = 8
N_CORE = N // N_CORES  # 12500 database rows per core
QB = B // 128  # 2 query blocks of 128

T_SM = 0.4  # softmax screen temperature
U_PAD = 15.0  # safety pad on the per-query shift estimate

# X tile widths per core: 2*1024 + 5*2048 + 224 = 12512 (12 pad cols)
TILE_W = [1024, 1024, 2048, 2048, 2048, 2048, 2048, 224]
N_PAD = sum(TILE_W)  # 12512
TILE_OFF = []
_o = 0
for _w in TILE_W:
    TILE_OFF.append(_o)
    _o += _w

# Per-tile screen split: DVE windowed max on [0:dw), ACT sumexp on
# [dw:w). Balanced for DVE@0.96GHz vs ACT@1.2GHz + 285ns accum-read.
def _tile_split(w):
    if w == 2048:
        return 1088, 64  # 17 windows of 64
    if w == 1024:
        return 640, 64  # 10 windows of 64
    if w == 224:
        return 224, 32  # 7 windows of 32, no ACT part
    raise AssertionError(w)


# Static unit table (device staging order). Entries:
#   (kind, qb, col0, width, valid_width)
# D units first in (tile, qb, window) order interleaved with A units is
# NOT needed -- we keep two separate staging tensors out_d / out_a and
# index them independently, each in (tile, qb) processing order.
def _unit_tables():
    units_d = []
    units_a = []
    for t, w in enumerate(TILE_W):
        dw, win = _tile_split(w)
        for qb in range(QB):
            for j in range(dw // win):
                col0 = TILE_OFF[t] + j * win
                vw = max(0, min(win, N_CORE - col0))
                units_d.append(("D", qb, col0, win, vw))
            if dw < w:
                col0 = TILE_OFF[t] + dw
                aw = w - dw
                vw = max(0, min(aw, N_CORE - col0))
                units_a.append(("A", qb, col0, aw, vw))
    return units_d, units_a


_UNITS_D, _UNITS_A = _unit_tables()
OUTW_D = len(_UNITS_D)  # 224
OUTW_A = len(_UNITS_A)  # 14
# out_d is DMA'd in two pieces; split at the unit count covering tiles
# t0..t4 (both qbs).
OUTD_SPLIT = sum(
    (_tile_split(w)[0] // _tile_split(w)[1]) * QB for w in TILE_W[:5]
)  # 142

_compiled = {}
LAST_EXEC_NS = None


def _build_kernel():
    nc = bacc.Bacc(
        "TRN2", target_bir_lowering=False, debug=False, num_devices=N_CORES
    )
    f32 = mybir.dt.float32
    bf16 = mybir.dt.bfloat16

    xt = nc.dram_tensor("xt", [D, N_PAD], bf16, kind="ExternalInput").ap()
    tq2_in = nc.dram_tensor("tq2", [D, B], bf16, kind="ExternalInput").ap()
    negb_in = nc.dram_tensor("negb", [128, QB], f32, kind="ExternalInput").ap()
    cd_a = nc.dram_tensor("cd_a", [128, OUTD_SPLIT], f32, kind="ExternalOutput").ap()
    cd_b = nc.dram_tensor(
        "cd_b", [128, OUTW_D - OUTD_SPLIT], f32, kind="ExternalOutput"
    ).ap()
    ca = nc.dram_tensor("ca", [128, OUTW_A], f32, kind="ExternalOutput").ap()

    with tile.TileContext(nc) as tc:
        with (
            tc.tile_pool(name="const", bufs=1) as cpool,
            tc.tile_pool(name="escr", bufs=2) as epool,
            tc.tile_pool(name="psum", bufs=2, space="PSUM") as pspool,
        ):
            # Tiny memset source for the exp-table preload + PE warmup.
            dummy = cpool.tile([128, 4], f32, name="dummy")
            nc.gpsimd.memset(dummy[:], 0.0)
            warm = cpool.tile([128, 512], bf16, name="warm")
            nc.gpsimd.memset(warm[:], 0.01)

            # Input DMAs: first X tile first so matmuls start ASAP.
            xts = []
            for t, w in enumerate(TILE_W):
                xts.append(cpool.tile([D, w], bf16, name=f"x{t}"))
            tq2 = cpool.tile([D, B], bf16, name="tq2s")
            negb = cpool.tile([128, QB], f32, name="negbs")
            nc.sync.dma_start(xts[0][:], xt[:, 0 : TILE_W[0]])
            nc.sync.dma_start(tq2[:], tq2_in[:])
            nc.sync.dma_start(negb[:], negb_in[:])
            for t in range(1, len(TILE_W)):
                nc.sync.dma_start(
                    xts[t][:], xt[:, TILE_OFF[t] : TILE_OFF[t] + TILE_W[t]]
                )

            out_da = cpool.tile([128, OUTD_SPLIT], f32, name="out_da")
            out_db = cpool.tile([128, OUTW_D - OUTD_SPLIT], f32, name="out_db")
            out_a = cpool.tile([128, OUTW_A], f32, name="out_a")

            # Preload the exp table set while DMAs run (ScalarE is idle).
            pre = epool.tile([128, 960], bf16, tag="e", name="pre")
            nc.scalar.activation(
                pre[:, 0:4], dummy[:], mybir.ActivationFunctionType.Exp,
                bias=0.0, scale=1.0,
            )

            # PE warmup: 2 matmuls on memset data into a scratch psum
            # tile, overlapped with the X DMA head (starts HAM ramp).
            ps_w = pspool.tile([128, 2048], f32, tag="ps", name="ps_warm")
            for i in range(2):
                nc.tensor.matmul(
                    ps_w[:, i * 512 : i * 512 + 512],
                    warm[:, 0:128],
                    warm[:, 0:512],
                    start=True,
                    stop=True,
                )

            dcol = 0
            acol = 0
            for t, w in enumerate(TILE_W):
                dw, win = _tile_split(w)
                nwin = dw // win
                for qb in range(QB):
                    ps = pspool.tile([128, 2048], f32, tag="ps", name=f"ps{t}_{qb}")
                    lhs = tq2[:, qb * 128 : (qb + 1) * 128]
                    for h0 in range(0, w, 512):
                        hw = min(512, w - h0)
                        nc.tensor.matmul(
                            ps[:, h0 : h0 + hw],
                            lhs,
                            xts[t][:, h0 : h0 + hw],
                            start=True,
                            stop=True,
                        )
                    # DVE windowed max over [0:dw)
                    out_t = out_da if dcol < OUTD_SPLIT else out_db
                    oc = dcol if dcol < OUTD_SPLIT else dcol - OUTD_SPLIT
                    nc.vector.tensor_reduce(
                        out_t[:, oc : oc + nwin],
                        ps[:, 0:dw].rearrange("p (a b) -> p a b", b=win),
                        axis=mybir.AxisListType.X,
                        op=mybir.AluOpType.max,
                    )
                    dcol += nwin
                    # ScalarE exp-sumexp over [dw:w)
                    if dw < w:
                        aw = w - dw
                        escr = epool.tile([128, 960], bf16, tag="e", name=f"e{t}_{qb}")
                        nc.scalar.activation(
                            escr[:, 0:aw],
                            ps[:, dw:w],
                            mybir.ActivationFunctionType.Exp,
                            bias=negb[:, qb : qb + 1],
                            scale=1.0 / T_SM,
                            accum_out=out_a[:, acol : acol + 1],
                        )
                        acol += 1
                # Overlap the first out_d piece with the tail tiles.
                if dcol == OUTD_SPLIT:
                    nc.sync.dma_start(cd_a[:], out_da[:])

            assert dcol == OUTW_D and acol == OUTW_A
            nc.sync.dma_start(cd_b[:], out_db[:])
            nc.sync.dma_start(ca[:], out_a[:])

    nc.compile()
    return nc


def _get_compiled():
    if "nc" not in _compiled:
        _compiled["nc"] = _build_kernel()
    return _compiled["nc"]


def _softmax_f32(x):
    x = x.astype(np.float32)
    m = np.max(x, axis=1, keepdims=True)
    e = np.exp(x - m)
    return e / np.sum(e, axis=1, keepdims=True)


def kernel(q_batch, q_indices, X, W, pre_indices, pre_weights):
    q_batch = np.asarray(q_batch, dtype=np.float32)
    X = np.asarray(X, dtype=np.float32)
    W = np.asarray(W, dtype=np.float32)
    q_indices = np.asarray(q_indices)
    pre_indices = np.asarray(pre_indices)
    pre_weights = np.asarray(pre_weights, dtype=np.float32)

    # ---- host prep: xnorm-sort the database, shard, quantize ---------------
    xnorm = np.sum(X * X, axis=1, dtype=np.float32)  # [N]
    order = np.argsort(xnorm, kind="stable")  # ascending
    Xs = np.ascontiguousarray(X[order])  # [N, D] sorted
    xn_s = xnorm[order]  # [N]
    tq2m = 2.0 * (q_batch @ W)  # [B, D] fp32
    tq2_dev = np.ascontiguousarray(tq2m.T.astype(ml_dtypes.bfloat16))  # [D, B]

    # per-query shift U_q: max inner-product score over a subsample + pad
    sub = Xs[:: max(1, N // 4096)]
    U_q = (tq2m @ sub.T).max(axis=1).astype(np.float32) + np.float32(U_PAD)
    negb = np.ascontiguousarray(
        (-U_q / np.float32(T_SM)).reshape(QB, 128).T.astype(np.float32)
    )  # [128, QB]

    xs_t = Xs.T.astype(ml_dtypes.bfloat16)  # [D, N]
    nc = _get_compiled()
    in_maps = []
    for c in range(N_CORES):
        xt_c = np.zeros((D, N_PAD), dtype=ml_dtypes.bfloat16)
        xt_c[:, 0:N_CORE] = xs_t[:, c * N_CORE : (c + 1) * N_CORE]
        in_maps.append(
            {"xt": np.ascontiguousarray(xt_c), "tq2": tq2_dev, "negb": negb}
        )
    res = run_bass_kernel_spmd(nc, in_maps, core_ids=list(range(N_CORES)))
    global LAST_EXEC_NS
    if res.exec_time_ns is not None:
        LAST_EXEC_NS = res.exec_time_ns

    # dev_vals: [128, N_CORES * (OUTW_D + OUTW_A)]
    outs = [
        np.concatenate(
            [res.results[c]["cd_a"], res.results[c]["cd_b"], res.results[c]["ca"]],
            axis=1,
        )
        for c in range(N_CORES)
    ]
    dev_vals = np.concatenate([o.astype(np.float64) for o in outs], axis=1)

    # ---- build per-unit bounds --------------------------------------------
    units = _UNITS_D + _UNITS_A
    nu = len(units)
    u_core = np.repeat(np.arange(N_CORES), nu)
    u_col0 = np.tile(np.array([u[2] for u in units]), N_CORES) + u_core * N_CORE
    u_w = np.tile(np.array([u[3] for u in units], dtype=np.float64), N_CORES)
    u_vw = np.tile(np.array([u[4] for u in units]), N_CORES)
    u_kind = np.tile(np.array([u[0] for u in units]), N_CORES)
    u_qb = np.tile(np.array([u[1] for u in units]), N_CORES)

    lnT = np.float64(T_SM)
    col0 = u_col0
    colend = u_col0 + u_vw - 1
    valid = u_vw > 0
    xmin2 = np.where(valid, xn_s[np.clip(col0, 0, N - 1)], 0.0)
    xmax2 = np.where(valid, xn_s[np.clip(colend, 0, N - 1)], 0.0)

    delta = 0.5  # device-vs-exact score deviation budget
    is_pool = u_kind == "D"
    is_act = ~is_pool
    partial = u_vw < np.array([u[3] for u in units] * N_CORES)

    ub = np.full((B, N_CORES * nu), -np.inf, dtype=np.float64)
    lb = np.full((B, N_CORES * nu), -np.inf, dtype=np.float64)
    rows = np.arange(B)[:, None]

    for _attempt in range(5):
        for qb in range(QB):
            qsl = slice(qb * 128, (qb + 1) * 128)
            m = u_qb == qb
            mp = m & is_pool & valid
            ma = m & is_act & valid
            dv = dev_vals[:, mp]
            ub_q = np.full((128, nu * N_CORES), -np.inf)
            lb_q = np.full((128, nu * N_CORES), -np.inf)
            ub_q[:, mp] = dv + delta - xmin2[mp][None, :]
            lb_q[:, mp] = dv - delta - xmax2[mp][None, :]
            dva = dev_vals[:, ma]
            with np.errstate(divide="ignore"):
                lns = np.log(dva)
            Uq = U_q[qsl].astype(np.float64)[:, None]
            ub_q[:, ma] = Uq + lnT * lns + delta - xmin2[ma][None, :]
            lb_q[:, ma] = (
                Uq + lnT * (lns - np.log(u_w[ma][None, :])) - delta - xmax2[ma][None, :]
            )
            lb_q[:, m & partial] = -np.inf
            ub[qsl] = ub_q
            lb[qsl] = lb_q

        t_q = -np.partition(-lb, K - 1, axis=1)[:, K - 1]  # [B]
        hits = ub >= t_q[:, None]  # [B, NU]

        # ---- exact rescan of hit units ------------------------------------
        t_rescan = time.time()
        cand_q = [[] for _ in range(B)]
        max_dev = 0.0
        act_viol = 0.0
        n_rescan_cols = 0
        hit_units = np.nonzero(hits.any(axis=0))[0]
        for u in hit_units:
            vw = u_vw[u]
            if vw == 0:
                continue
            qb = u_qb[u]
            qhit = np.nonzero(hits[:, u])[0]
            qs = qhit[(qhit >= qb * 128) & (qhit < (qb + 1) * 128)]
            if qs.size == 0:
                continue
            pr = qs - qb * 128
            g0 = u_col0[u]
            Xc = Xs[g0 : g0 + vw]
            Sp = tq2m[qs] @ Xc.T  # [nq, vw] exact s'
            S = Sp - xn_s[g0 : g0 + vw][None, :]
            if u_kind[u] == "D" and not partial[u]:
                dmax = np.max(np.abs(Sp.max(axis=1) - dev_vals[pr, u]))
                max_dev = max(max_dev, float(dmax))
            elif u_kind[u] == "A":
                ub_sp = (
                    U_q[qs].astype(np.float64)
                    + lnT * np.log(np.maximum(dev_vals[pr, u], 1e-300))
                    + delta
                )
                act_viol = max(act_viol, float(np.max(Sp.max(axis=1) - ub_sp)))
            n_rescan_cols += int(qs.size) * int(vw)
            rr, cc = np.nonzero(S >= t_q[qs, None])
            svals = S[rr, cc]
            gidx = order[g0 + cc]
            qq = qs[rr]
            for r_i in range(rr.size):
                cand_q[qq[r_i]].append((float(svals[r_i]), int(gidx[r_i])))

        ok = (
            max_dev <= delta - 0.05
            and act_viol <= 0.02
            and all(len(lst) >= K for lst in cand_q)
        )
        if MERGE_DEBUG:
            print(
                f"merge attempt={_attempt} delta={delta:.3f} "
                f"max_dev={max_dev:.3f} act_viol={act_viol:.3f} "
                f"hit_units={hit_units.size} rescan_cols={n_rescan_cols} "
                f"ncand_min={min(len(l) for l in cand_q)} "
                f"rescan_t={time.time() - t_rescan:.2f}s ok={ok}",
                file=sys.stderr,
            )
        if ok:
            break
        delta = max(2.0 * delta, 2.5 * max_dev + 0.1)

    post_idx = np.empty((B, K), dtype=np.int64)
    for q in range(B):
        lst = cand_q[q]
        assert len(lst) >= K, f"query {q}: only {len(lst)} candidates"
        lst.sort(key=lambda vc: (-vc[0], vc[1]))
        post_idx[q] = [gi for _, gi in lst[:K]]

    # ---- final loss (tiny), mirroring the reference math ------------------
    T_qm = q_batch @ W  # [B, D] fp32
    X_nb = X[post_idx]  # [B, K, D]
    diff = T_qm[:, None, :] - X_nb
    l2 = np.sum(diff * diff, axis=-1, dtype=np.float32)  # [B, K]
    post_w = _softmax_f32(-l2 / np.float32(TAU))  # [B, K]

    pre_idx_b = pre_indices[q_indices]  # [B, K]
    pre_w_b = pre_weights[q_indices]  # [B, K]

    p_dense = np.zeros((B, N), np.float32)
    p_dense[rows, pre_idx_b] = pre_w_b
    q_dense = np.zeros((B, N), np.float32)
    q_dense[rows, post_idx] = post_w
    union = (p_dense > 0) | (q_dense > 0)
    p = np.where(union, np.maximum(p_dense, np.float32(EPS)), np.float32(0.0))
    p = p / p.sum(axis=1, keepdims=True)
    q = np.where(union, np.maximum(q_dense, np.float32(EPS)), np.float32(0.0))
    q = q / q.sum(axis=1, keepdims=True)
    logp = np.where(union, np.log(np.maximum(p, np.float32(1e-20))), np.float32(0.0))
    logq = np.where(union, np.log(np.maximum(q, np.float32(1e-20))), np.float32(0.0))
    kl = np.sum(np.where(union, p * (logp - logq), np.float32(0.0)), axis=1)
    loss_knn = np.float32(np.mean(kl))
    loss_reg = np.float32(0.5) * np.float32(np.sum(W * W))
    total_loss = np.float32(BETA) * loss_knn + np.float32(LAMB) * loss_reg
    return (
        np.float32(total_loss),
        np.float32(0.0),
        np.float32(loss_knn),
    )


# revision 4
# speedup vs baseline: 1.0369x; 1.0369x over previous
"""Trainium2 Bass kernel for nn_CustomLoss_11630771438153 (retrieval_knn).

Strategy v4: the kernel is DMA-in bound (measured ~92 GB/s effective per
core), so the database is shipped as FP8 (e4m3): 1.25 MB per core
instead of 3.2 MB bf16. Each core holds 1/8 of the xnorm-sorted
database (12500 rows, zero-padded to 12512) and computes raw
inner-product scores s' = tq2.T @ X_shard on the TensorEngine (fp8 x
fp8 -> fp32 PSUM). Every score element is screened by the two engines
that can read PSUM, balanced to finish together:

  - DVE `tensor_reduce` max over 64-col windows -> tight bounds.
  - ScalarE `activation(Exp, bias=-U_q/T, scale=1/T, accum_out)` ->
    log-sum-exp bounds per group.

Query blocks are processed per-X-tile interleaved so each X tile is
consumed right after its DMA; X DMAs are split across the Sync and
GpSimd issue queues for more outstanding transfers. Exp table load and
a 4-matmul HAM warmup overlap the DMA head. Outputs are small f32
tensors DMA'd in two overlapping pieces.

Host: per-unit bounds -> t_q = 16th best lower bound -> exact rescan of
hit units (numpy) with a probed fp8 deviation budget delta and
escalation on validation failure -> exact loss.
"""

import os
import sys
import time

sys.path.insert(0, "/opt/trn_rl_repo")

MERGE_DEBUG = bool(os.environ.get("MERGE_DEBUG"))

import ml_dtypes
import numpy as np

import concourse.tile as tile
from concourse import bacc, mybir
from concourse.bass_utils import run_bass_kernel_spmd

B = 256
D = 128
N = 100000
K = 16
TAU = 0.1
BETA = 1.0
LAMB = 1e-4
EPS = 1e-8

N_CORES = 8
N_CORE = N // N_CORES  # 12500
QB = B // 128

T_SM = 0.4
U_PAD = 15.0

FP8 = ml_dtypes.float8_e4m3

# X tile widths per core: 512 + 5*2048 + 1024 + 512 + 224 = 12512
TILE_W = [512, 2048, 2048, 2048, 2048, 2048, 1024, 512, 224]
N_PAD = sum(TILE_W)
TILE_OFF = []
_o = 0
for _w in TILE_W:
    TILE_OFF.append(_o)
    _o += _w

# Per-tile screen split: DVE windowed max on [0:dw) with window `win`,
# ScalarE sumexp on [dw:w). Balanced for DVE@0.96 vs ACT@1.2GHz+285ns.
def _tile_split(w):
    if w == 2048:
        return 896, 64  # 14 windows
    if w == 1024:
        return 640, 64  # 10 windows
    if w == 512:
        return 512, 64  # 8 windows, DVE only
    if w == 224:
        return 224, 32  # 7 windows, DVE only
    raise AssertionError(w)


def _unit_tables():
    units_d = []
    units_a = []
    for t, w in enumerate(TILE_W):
        dw, win = _tile_split(w)
        for qb in range(QB):
            for j in range(dw // win):
                col0 = TILE_OFF[t] + j * win
                vw = max(0, min(win, N_CORE - col0))
                units_d.append(("D", qb, col0, win, vw))
            if dw < w:
                col0 = TILE_OFF[t] + dw
                aw = w - dw
                vw = max(0, min(aw, N_CORE - col0))
                units_a.append(("A", qb, col0, aw, vw))
    return units_d, units_a


_UNITS_D, _UNITS_A = _unit_tables()
OUTW_D = len(_UNITS_D)  # 206
OUTW_A = len(_UNITS_A)  # 12
# First out_d piece covers tiles t0..t5 (both qbs), DMA'd mid-kernel.
OUTD_SPLIT = sum(
    (_tile_split(w)[0] // _tile_split(w)[1]) * QB for w in TILE_W[:6]
)  # 156

_compiled = {}
LAST_EXEC_NS = None


def _build_kernel():
    nc = bacc.Bacc(
        "TRN2", target_bir_lowering=False, debug=False, num_devices=N_CORES
    )
    f32 = mybir.dt.float32
    bf16 = mybir.dt.bfloat16
    fp8 = mybir.dt.float8e4

    xt = nc.dram_tensor("xt", [D, N_PAD], fp8, kind="ExternalInput").ap()
    tq2_in = nc.dram_tensor("tq2", [D, B], fp8, kind="ExternalInput").ap()
    negb_in = nc.dram_tensor("negb", [128, QB], f32, kind="ExternalInput").ap()
    cd_a = nc.dram_tensor("cd_a", [128, OUTD_SPLIT], f32, kind="ExternalOutput").ap()
    cd_b = nc.dram_tensor(
        "cd_b", [128, OUTW_D - OUTD_SPLIT], f32, kind="ExternalOutput"
    ).ap()
    ca = nc.dram_tensor("ca", [128, OUTW_A], f32, kind="ExternalOutput").ap()

    with tile.TileContext(nc) as tc:
        with (
            tc.tile_pool(name="const", bufs=1) as cpool,
            tc.tile_pool(name="escr", bufs=2) as epool,
            tc.tile_pool(name="psum", bufs=2, space="PSUM") as pspool,
        ):
            dummy = cpool.tile([128, 4], f32, name="dummy")
            nc.gpsimd.memset(dummy[:], 0.0)
            warm = cpool.tile([128, 512], bf16, name="warm")
            nc.gpsimd.memset(warm[:], 0.01)

            xts = [
                cpool.tile([D, w], fp8, name=f"x{t}")
                for t, w in enumerate(TILE_W)
            ]
            tq2 = cpool.tile([D, B], fp8, name="tq2s")
            negb = cpool.tile([128, QB], f32, name="negbs")

            # Input DMAs: first tile + tq2/negb first; big tiles split in
            # halves across the Sync and GpSimd issue queues.
            nc.sync.dma_start(xts[0][:], xt[:, 0 : TILE_W[0]])
            nc.gpsimd.dma_start(tq2[:], tq2_in[:])
            nc.gpsimd.dma_start(negb[:], negb_in[:])
            for t in range(1, len(TILE_W)):
                w = TILE_W[t]
                o = TILE_OFF[t]
                if w >= 1024:
                    h = w // 2
                    nc.sync.dma_start(xts[t][:, 0:h], xt[:, o : o + h])
                    nc.gpsimd.dma_start(xts[t][:, h:w], xt[:, o + h : o + w])
                else:
                    nc.sync.dma_start(xts[t][:], xt[:, o : o + w])

            out_da = cpool.tile([128, OUTD_SPLIT], f32, name="out_da")
            out_db = cpool.tile([128, OUTW_D - OUTD_SPLIT], f32, name="out_db")
            out_a = cpool.tile([128, OUTW_A], f32, name="out_a")

            # Preload the exp table set while DMAs run.
            pre = epool.tile([128, 1152], bf16, tag="e", name="pre")
            nc.scalar.activation(
                pre[:, 0:4], dummy[:], mybir.ActivationFunctionType.Exp,
                bias=0.0, scale=1.0,
            )

            # HAM warmup: 4 back-to-back matmuls on memset data.
            ps_w = pspool.tile([128, 2048], f32, tag="ps", name="ps_warm")
            for i in range(4):
                nc.tensor.matmul(
                    ps_w[:, (i % 4) * 512 : (i % 4) * 512 + 512],
                    warm[:, 0:128],
                    warm[:, 0:512],
                    start=True,
                    stop=True,
                )

            dcol = 0
            acol = 0
            for t, w in enumerate(TILE_W):
                dw, win = _tile_split(w)
                nwin = dw // win
                for qb in range(QB):
                    ps = pspool.tile([128, 2048], f32, tag="ps", name=f"ps{t}_{qb}")
                    lhs = tq2[:, qb * 128 : (qb + 1) * 128]
                    for h0 in range(0, w, 512):
                        hw = min(512, w - h0)
                        nc.tensor.matmul(
                            ps[:, h0 : h0 + hw],
                            lhs,
                            xts[t][:, h0 : h0 + hw],
                            start=True,
                            stop=True,
                        )
                    out_t = out_da if dcol < OUTD_SPLIT else out_db
                    oc = dcol if dcol < OUTD_SPLIT else dcol - OUTD_SPLIT
                    nc.vector.tensor_reduce(
                        out_t[:, oc : oc + nwin],
                        ps[:, 0:dw].rearrange("p (a b) -> p a b", b=win),
                        axis=mybir.AxisListType.X,
                        op=mybir.AluOpType.max,
                    )
                    dcol += nwin
                    if dw < w:
                        aw = w - dw
                        escr = epool.tile(
                            [128, 1152], bf16, tag="e", name=f"e{t}_{qb}"
                        )
                        nc.scalar.activation(
                            escr[:, 0:aw],
                            ps[:, dw:w],
                            mybir.ActivationFunctionType.Exp,
                            bias=negb[:, qb : qb + 1],
                            scale=1.0 / T_SM,
                            accum_out=out_a[:, acol : acol + 1],
                        )
                        acol += 1
                if dcol == OUTD_SPLIT:
                    nc.sync.dma_start(cd_a[:], out_da[:])

            assert dcol == OUTW_D and acol == OUTW_A
            nc.sync.dma_start(cd_b[:], out_db[:])
            nc.sync.dma_start(ca[:], out_a[:])

    nc.compile()
    return nc


def _get_compiled():
    if "nc" not in _compiled:
        _compiled["nc"] = _build_kernel()
    return _compiled["nc"]


def _softmax_f32(x):
    x = x.astype(np.float32)
    m = np.max(x, axis=1, keepdims=True)
    e = np.exp(x - m)
    return e / np.sum(e, axis=1, keepdims=True)


def kernel(q_batch, q_indices, X, W, pre_indices, pre_weights):
    q_batch = np.asarray(q_batch, dtype=np.float32)
    X = np.asarray(X, dtype=np.float32)
    W = np.asarray(W, dtype=np.float32)
    q_indices = np.asarray(q_indices)
    pre_indices = np.asarray(pre_indices)
    pre_weights = np.asarray(pre_weights, dtype=np.float32)

    # ---- host prep: xnorm-sort, shard, quantize to fp8 --------------------
    xnorm = np.sum(X * X, axis=1, dtype=np.float32)
    order = np.argsort(xnorm, kind="stable")
    Xs = np.ascontiguousarray(X[order])
    xn_s = xnorm[order]
    tq2m = 2.0 * (q_batch @ W)  # [B, D] fp32
    tq2_q8 = tq2m.T.astype(FP8)  # [D, B] fp8
    tq2_deq = tq2_q8.astype(np.float32)

    sub = Xs[:: max(1, N // 4096)]
    U_q = (tq2m @ sub.T).max(axis=1).astype(np.float32) + np.float32(U_PAD)
    negb = np.ascontiguousarray(
        (-U_q / np.float32(T_SM)).reshape(QB, 128).T.astype(np.float32)
    )

    xs_t8 = Xs.T.astype(FP8)  # [D, N] fp8
    xs_deq = None  # lazily built for rescan validation probes

    # fp8 deviation probe: device-score vs exact-score on spread columns.
    probe = np.arange(0, N, max(1, N // 384))[:384]
    Sdev_p = tq2_deq.T @ xs_t8[:, probe].astype(np.float32)  # [B, np]
    Sex_p = tq2m @ Xs[probe].T
    delta = float(np.max(np.abs(Sdev_p - Sex_p))) * 1.35 + 0.25
    if MERGE_DEBUG:
        print(f"fp8 probe delta = {delta:.3f}", file=sys.stderr)

    nc = _get_compiled()
    in_maps = []
    for c in range(N_CORES):
        xt_c = np.zeros((D, N_PAD), dtype=FP8)
        xt_c[:, 0:N_CORE] = xs_t8[:, c * N_CORE : (c + 1) * N_CORE]
        in_maps.append(
            {"xt": np.ascontiguousarray(xt_c), "tq2": tq2_q8, "negb": negb}
        )
    res = run_bass_kernel_spmd(nc, in_maps, core_ids=list(range(N_CORES)))
    global LAST_EXEC_NS
    if res.exec_time_ns is not None:
        LAST_EXEC_NS = res.exec_time_ns

    outs = [
        np.concatenate(
            [res.results[c]["cd_a"], res.results[c]["cd_b"], res.results[c]["ca"]],
            axis=1,
        )
        for c in range(N_CORES)
    ]
    dev_vals = np.concatenate([o.astype(np.float64) for o in outs], axis=1)

    # ---- per-unit bounds ---------------------------------------------------
    units = _UNITS_D + _UNITS_A
    nu = len(units)
    u_core = np.repeat(np.arange(N_CORES), nu)
    u_col0 = np.tile(np.array([u[2] for u in units]), N_CORES) + u_core * N_CORE
    u_w = np.tile(np.array([u[3] for u in units], dtype=np.float64), N_CORES)
    u_vw = np.tile(np.array([u[4] for u in units]), N_CORES)
    u_kind = np.tile(np.array([u[0] for u in units]), N_CORES)
    u_qb = np.tile(np.array([u[1] for u in units]), N_CORES)

    lnT = np.float64(T_SM)
    col0 = u_col0
    colend = u_col0 + u_vw - 1
    valid = u_vw > 0
    xmin2 = np.where(valid, xn_s[np.clip(col0, 0, N - 1)], 0.0)
    xmax2 = np.where(valid, xn_s[np.clip(colend, 0, N - 1)], 0.0)

    is_pool = u_kind == "D"
    is_act = ~is_pool
    partial = u_vw < np.array([u[3] for u in units] * N_CORES)

    ub = np.full((B, N_CORES * nu), -np.inf, dtype=np.float64)
    lb = np.full((B, N_CORES * nu), -np.inf, dtype=np.float64)
    rows = np.arange(B)[:, None]

    for _attempt in range(5):
        for qb in range(QB):
            qsl = slice(qb * 128, (qb + 1) * 128)
            m = u_qb == qb
            mp = m & is_pool & valid
            ma = m & is_act & valid
            dv = dev_vals[:, mp]
            ub_q = np.full((128, nu * N_CORES), -np.inf)
            lb_q = np.full((128, nu * N_CORES), -np.inf)
            ub_q[:, mp] = dv + delta - xmin2[mp][None, :]
            lb_q[:, mp] = dv - delta - xmax2[mp][None, :]
            dva = dev_vals[:, ma]
            with np.errstate(divide="ignore"):
                lns = np.log(dva)
            Uq = U_q[qsl].astype(np.float64)[:, None]
            ub_q[:, ma] = Uq + lnT * lns + delta - xmin2[ma][None, :]
            lb_q[:, ma] = (
                Uq + lnT * (lns - np.log(u_w[ma][None, :])) - delta - xmax2[ma][None, :]
            )
            lb_q[:, m & partial] = -np.inf
            ub[qsl] = ub_q
            lb[qsl] = lb_q

        t_q = -np.partition(-lb, K - 1, axis=1)[:, K - 1]
        hits = ub >= t_q[:, None]

        # ---- exact rescan of hit units ------------------------------------
        t_rescan = time.time()
        cand_q = [[] for _ in range(B)]
        max_dev = 0.0
        act_viol = 0.0
        n_rescan_cols = 0
        hit_units = np.nonzero(hits.any(axis=0))[0]
        for u in hit_units:
            vw = u_vw[u]
            if vw == 0:
                continue
            qb = u_qb[u]
            qhit = np.nonzero(hits[:, u])[0]
            qs = qhit[(qhit >= qb * 128) & (qhit < (qb + 1) * 128)]
            if qs.size == 0:
                continue
            pr = qs - qb * 128
            g0 = u_col0[u]
            Xc = Xs[g0 : g0 + vw]
            Sp = tq2m[qs] @ Xc.T  # exact s'
            S = Sp - xn_s[g0 : g0 + vw][None, :]
            if u_kind[u] == "D" and not partial[u]:
                dmax = np.max(np.abs(Sp.max(axis=1) - dev_vals[pr, u]))
                max_dev = max(max_dev, float(dmax))
            elif u_kind[u] == "A":
                ub_sp = (
                    U_q[qs].astype(np.float64)
                    + lnT * np.log(np.maximum(dev_vals[pr, u], 1e-300))
                    + delta
                )
                act_viol = max(act_viol, float(np.max(Sp.max(axis=1) - ub_sp)))
            n_rescan_cols += int(qs.size) * int(vw)
            rr, cc = np.nonzero(S >= t_q[qs, None])
            svals = S[rr, cc]
            gidx = order[g0 + cc]
            qq = qs[rr]
            for r_i in range(rr.size):
                cand_q[qq[r_i]].append((float(svals[r_i]), int(gidx[r_i])))

        ok = (
            max_dev <= delta - 0.05
            and act_viol <= 0.02
            and all(len(lst) >= K for lst in cand_q)
        )
        if MERGE_DEBUG:
            print(
                f"merge attempt={_attempt} delta={delta:.3f} "
                f"max_dev={max_dev:.3f} act_viol={act_viol:.3f} "
                f"hit_units={hit_units.size} rescan_cols={n_rescan_cols} "
                f"ncand_min={min(len(l) for l in cand_q)} "
                f"rescan_t={time.time() - t_rescan:.2f}s ok={ok}",
                file=sys.stderr,
            )
        if ok:
            break
        delta = max(1.5 * delta, 2.0 * max_dev + 0.1)

    post_idx = np.empty((B, K), dtype=np.int64)
    for q in range(B):
        lst = cand_q[q]
        assert len(lst) >= K, f"query {q}: only {len(lst)} candidates"
        lst.sort(key=lambda vc: (-vc[0], vc[1]))
        post_idx[q] = [gi for _, gi in lst[:K]]

    # ---- final loss, mirroring the reference math -------------------------
    T_qm = q_batch @ W
    X_nb = X[post_idx]
    diff = T_qm[:, None, :] - X_nb
    l2 = np.sum(diff * diff, axis=-1, dtype=np.float32)
    post_w = _softmax_f32(-l2 / np.float32(TAU))

    pre_idx_b = pre_indices[q_indices]
    pre_w_b = pre_weights[q_indices]

    p_dense = np.zeros((B, N), np.float32)
    p_dense[rows, pre_idx_b] = pre_w_b
    q_dense = np.zeros((B, N), np.float32)
    q_dense[rows, post_idx] = post_w
    union = (p_dense > 0) | (q_dense > 0)
    p = np.where(union, np.maximum(p_dense, np.float32(EPS)), np.float32(0.0))
    p = p / p.sum(axis=1, keepdims=True)
    q = np.where(union, np.maximum(q_dense, np.float32(EPS)), np.float32(0.0))
    q = q / q.sum(axis=1, keepdims=True)
    logp = np.where(union, np.log(np.maximum(p, np.float32(1e-20))), np.float32(0.0))
    logq = np.where(union, np.log(np.maximum(q, np.float32(1e-20))), np.float32(0.0))
    kl = np.sum(np.where(union, p * (logp - logq), np.float32(0.0)), axis=1)
    loss_knn = np.float32(np.mean(kl))
    loss_reg = np.float32(0.5) * np.float32(np.sum(W * W))
    total_loss = np.float32(BETA) * loss_knn + np.float32(LAMB) * loss_reg
    return (
        np.float32(total_loss),
        np.float32(0.0),
        np.float32(loss_knn),
    )


# revision 5
# speedup vs baseline: 1.2867x; 1.2409x over previous
"""Trainium2 Bass kernel for nn_CustomLoss_11630771438153 (retrieval_knn).

Strategy v5: the kernel is limited by DMA-in bandwidth and by the two
PSUM-reading engines (DVE 0.96 GHz + ScalarE 1.2 GHz, 1 elem/cycle/lane
each), so:

  - The xnorm-sorted database is shipped FP8 (e4m3): 1.25 MB/core.
  - Scores s' = tq2.T @ X_shard are computed fp8 x fp8 -> fp32 PSUM in
    1024-col tiles, two query blocks per X tile, with a 4-deep PSUM
    pipeline ([128,1024] x 4 buffers = 8 banks) so matmul refills never
    sit on the screens' critical path.
  - Each screen tile is consumed whole by ONE engine, alternating:
    DVE `tensor_reduce` max over 64-col windows (tight bounds), or
    ScalarE `activation(Exp, bias=-U_q/T, scale=1/T, accum_out)`
    (log-sum-exp bounds). Assignment alternates per X-tile and qb so
    both engines stay saturated and each query block gets both kinds.
  - Exp table load overlaps the DMA head; X DMAs are split across the
    Sync and GpSimd issue queues; outputs are small f32 tensors DMA'd
    in two overlapping pieces. No warmup matmuls (PE cold costs less
    than delaying the first screens).

Host: per-unit bounds -> t_q = 16th best lower bound -> exact rescan of
hit units with a probed fp8 deviation budget delta (escalation on
validation failure) -> exact loss.
"""

import os
import sys
import time

sys.path.insert(0, "/opt/trn_rl_repo")

MERGE_DEBUG = bool(os.environ.get("MERGE_DEBUG"))

import ml_dtypes
import numpy as np

import concourse.tile as tile
from concourse import bacc, mybir
from concourse.bass_utils import run_bass_kernel_spmd

B = 256
D = 128
N = 100000
K = 16
TAU = 0.1
BETA = 1.0
LAMB = 1e-4
EPS = 1e-8

N_CORES = 8
N_CORE = N // N_CORES  # 12500
QB = B // 128

T_SM = 0.4
U_PAD = 15.0

FP8 = ml_dtypes.float8_e4m3

# X tile widths per core: 512 + 11*1024 + 512 + 224 = 12512
TILE_W = [512] + [1024] * 11 + [512, 224]
N_PAD = sum(TILE_W)
TILE_OFF = []
_o = 0
for _w in TILE_W:
    TILE_OFF.append(_o)
    _o += _w
NT = len(TILE_W)


def _kind(t, qb):
    """Screen kind for (X tile, query block): 'D' (DVE windowed max) or
    'A' (ScalarE sumexp). 224-tail is always D (win32)."""
    if TILE_W[t] == 224:
        return "D"
    if t % 2 == 0:
        return "D" if qb == 0 else "A"
    return "A" if qb == 0 else "D"


def _win(w):
    return 32 if w == 224 else 64


def _unit_tables():
    units_d = []
    units_a = []
    for t in range(NT):
        w = TILE_W[t]
        win = _win(w)
        for qb in range(QB):
            if _kind(t, qb) == "D":
                for j in range(w // win):
                    col0 = TILE_OFF[t] + j * win
                    vw = max(0, min(win, N_CORE - col0))
                    units_d.append(("D", qb, col0, win, vw))
            else:
                col0 = TILE_OFF[t]
                vw = max(0, min(w, N_CORE - col0))
                units_a.append(("A", qb, col0, w, vw))
    return units_d, units_a


_UNITS_D, _UNITS_A = _unit_tables()
OUTW_D = len(_UNITS_D)  # 11*16 + 2*8 + 2*7 = 206
OUTW_A = len(_UNITS_A)  # 13
# First out_d piece: D units of X tiles t0..t8 (DMA'd mid-kernel).
OUTD_SPLIT = sum(
    TILE_W[t] // _win(TILE_W[t])
    for t in range(9)
    for qb in range(QB)
    if _kind(t, qb) == "D"
)

_compiled = {}
LAST_EXEC_NS = None


def _build_kernel():
    nc = bacc.Bacc(
        "TRN2", target_bir_lowering=False, debug=False, num_devices=N_CORES
    )
    f32 = mybir.dt.float32
    bf16 = mybir.dt.bfloat16
    fp8 = mybir.dt.float8e4

    xt = nc.dram_tensor("xt", [D, N_PAD], fp8, kind="ExternalInput").ap()
    tq2_in = nc.dram_tensor("tq2", [D, B], fp8, kind="ExternalInput").ap()
    negb_in = nc.dram_tensor("negb", [128, QB], f32, kind="ExternalInput").ap()
    cd_a = nc.dram_tensor("cd_a", [128, OUTD_SPLIT], f32, kind="ExternalOutput").ap()
    cd_b = nc.dram_tensor(
        "cd_b", [128, OUTW_D - OUTD_SPLIT], f32, kind="ExternalOutput"
    ).ap()
    ca = nc.dram_tensor("ca", [128, OUTW_A], f32, kind="ExternalOutput").ap()

    with tile.TileContext(nc) as tc:
        with (
            tc.tile_pool(name="const", bufs=1) as cpool,
            tc.tile_pool(name="escr", bufs=2) as epool,
            tc.tile_pool(name="psum", bufs=4, space="PSUM") as pspool,
        ):
            dummy = cpool.tile([128, 4], f32, name="dummy")
            nc.gpsimd.memset(dummy[:], 0.0)

            xts = [
                cpool.tile([D, w], fp8, name=f"x{t}")
                for t, w in enumerate(TILE_W)
            ]
            tq2 = cpool.tile([D, B], fp8, name="tq2s")
            negb = cpool.tile([128, QB], f32, name="negbs")

            # Input DMAs: x0 + tq2/negb first, then the rest alternating
            # between the Sync and GpSimd issue queues.
            nc.sync.dma_start(xts[0][:], xt[:, 0 : TILE_W[0]])
            nc.gpsimd.dma_start(tq2[:], tq2_in[:])
            nc.gpsimd.dma_start(negb[:], negb_in[:])
            for t in range(1, NT):
                w = TILE_W[t]
                o = TILE_OFF[t]
                q = nc.sync if t % 2 == 1 else nc.gpsimd
                q.dma_start(xts[t][:], xt[:, o : o + w])

            out_da = cpool.tile([128, OUTD_SPLIT], f32, name="out_da")
            out_db = cpool.tile([128, OUTW_D - OUTD_SPLIT], f32, name="out_db")
            out_a = cpool.tile([128, OUTW_A], f32, name="out_a")

            # Preload the exp table set while DMAs run.
            pre = epool.tile([128, 1024], bf16, tag="e", name="pre")
            nc.scalar.activation(
                pre[:, 0:4], dummy[:], mybir.ActivationFunctionType.Exp,
                bias=0.0, scale=1.0,
            )

            dcol = 0
            acol = 0
            for t in range(NT):
                w = TILE_W[t]
                win = _win(w)
                for qb in range(QB):
                    ps = pspool.tile([128, 1024], f32, tag="ps", name=f"ps{t}_{qb}")
                    lhs = tq2[:, qb * 128 : (qb + 1) * 128]
                    for h0 in range(0, w, 512):
                        hw = min(512, w - h0)
                        nc.tensor.matmul(
                            ps[:, h0 : h0 + hw],
                            lhs,
                            xts[t][:, h0 : h0 + hw],
                            start=True,
                            stop=True,
                        )
                    if _kind(t, qb) == "D":
                        nwin = w // win
                        out_t = out_da if dcol < OUTD_SPLIT else out_db
                        oc = dcol if dcol < OUTD_SPLIT else dcol - OUTD_SPLIT
                        nc.vector.tensor_reduce(
                            out_t[:, oc : oc + nwin],
                            ps[:, 0:w].rearrange("p (a b) -> p a b", b=win),
                            axis=mybir.AxisListType.X,
                            op=mybir.AluOpType.max,
                        )
                        dcol += nwin
                    else:
                        escr = epool.tile(
                            [128, 1024], bf16, tag="e", name=f"e{t}_{qb}"
                        )
                        nc.scalar.activation(
                            escr[:, 0:w],
                            ps[:, 0:w],
                            mybir.ActivationFunctionType.Exp,
                            bias=negb[:, qb : qb + 1],
                            scale=1.0 / T_SM,
                            accum_out=out_a[:, acol : acol + 1],
                        )
                        acol += 1
                if dcol == OUTD_SPLIT and t < NT - 1:
                    nc.sync.dma_start(cd_a[:], out_da[:])

            assert dcol == OUTW_D and acol == OUTW_A, (dcol, acol)
            nc.sync.dma_start(cd_b[:], out_db[:])
            nc.sync.dma_start(ca[:], out_a[:])

    nc.compile()
    return nc


def _get_compiled():
    if "nc" not in _compiled:
        _compiled["nc"] = _build_kernel()
    return _compiled["nc"]


def _softmax_f32(x):
    x = x.astype(np.float32)
    m = np.max(x, axis=1, keepdims=True)
    e = np.exp(x - m)
    return e / np.sum(e, axis=1, keepdims=True)


def kernel(q_batch, q_indices, X, W, pre_indices, pre_weights):
    q_batch = np.asarray(q_batch, dtype=np.float32)
    X = np.asarray(X, dtype=np.float32)
    W = np.asarray(W, dtype=np.float32)
    q_indices = np.asarray(q_indices)
    pre_indices = np.asarray(pre_indices)
    pre_weights = np.asarray(pre_weights, dtype=np.float32)

    # ---- host prep: xnorm-sort, shard, quantize to fp8 --------------------
    xnorm = np.sum(X * X, axis=1, dtype=np.float32)
    order = np.argsort(xnorm, kind="stable")
    Xs = np.ascontiguousarray(X[order])
    xn_s = xnorm[order]
    tq2m = 2.0 * (q_batch @ W)  # [B, D] fp32
    tq2_q8 = tq2m.T.astype(FP8)  # [D, B] fp8
    tq2_deq = tq2_q8.astype(np.float32)

    sub = Xs[:: max(1, N // 4096)]
    U_q = (tq2m @ sub.T).max(axis=1).astype(np.float32) + np.float32(U_PAD)
    negb = np.ascontiguousarray(
        (-U_q / np.float32(T_SM)).reshape(QB, 128).T.astype(np.float32)
    )

    xs_t8 = Xs.T.astype(FP8)  # [D, N] fp8

    # fp8 deviation probe: device-score vs exact-score on spread columns.
    probe = np.arange(0, N, max(1, N // 384))[:384]
    Sdev_p = tq2_deq.T @ xs_t8[:, probe].astype(np.float32)
    Sex_p = tq2m @ Xs[probe].T
    delta = float(np.max(np.abs(Sdev_p - Sex_p))) * 1.35 + 0.25
    if MERGE_DEBUG:
        print(f"fp8 probe delta = {delta:.3f}", file=sys.stderr)

    nc = _get_compiled()
    in_maps = []
    for c in range(N_CORES):
        xt_c = np.zeros((D, N_PAD), dtype=FP8)
        xt_c[:, 0:N_CORE] = xs_t8[:, c * N_CORE : (c + 1) * N_CORE]
        in_maps.append(
            {"xt": np.ascontiguousarray(xt_c), "tq2": tq2_q8, "negb": negb}
        )
    res = run_bass_kernel_spmd(nc, in_maps, core_ids=list(range(N_CORES)))
    global LAST_EXEC_NS
    if res.exec_time_ns is not None:
        LAST_EXEC_NS = res.exec_time_ns

    outs = [
        np.concatenate(
            [res.results[c]["cd_a"], res.results[c]["cd_b"], res.results[c]["ca"]],
            axis=1,
        )
        for c in range(N_CORES)
    ]
    dev_vals = np.concatenate([o.astype(np.float64) for o in outs], axis=1)

    # ---- per-unit bounds ---------------------------------------------------
    units = _UNITS_D + _UNITS_A
    nu = len(units)
    u_core = np.repeat(np.arange(N_CORES), nu)
    u_col0 = np.tile(np.array([u[2] for u in units]), N_CORES) + u_core * N_CORE
    u_w = np.tile(np.array([u[3] for u in units], dtype=np.float64), N_CORES)
    u_vw = np.tile(np.array([u[4] for u in units]), N_CORES)
    u_kind = np.tile(np.array([u[0] for u in units]), N_CORES)
    u_qb = np.tile(np.array([u[1] for u in units]), N_CORES)

    lnT = np.float64(T_SM)
    col0 = u_col0
    colend = u_col0 + u_vw - 1
    valid = u_vw > 0
    xmin2 = np.where(valid, xn_s[np.clip(col0, 0, N - 1)], 0.0)
    xmax2 = np.where(valid, xn_s[np.clip(colend, 0, N - 1)], 0.0)

    is_pool = u_kind == "D"
    is_act = ~is_pool
    partial = u_vw < np.array([u[3] for u in units] * N_CORES)

    ub = np.full((B, N_CORES * nu), -np.inf, dtype=np.float64)
    lb = np.full((B, N_CORES * nu), -np.inf, dtype=np.float64)
    rows = np.arange(B)[:, None]

    for _attempt in range(5):
        for qb in range(QB):
            qsl = slice(qb * 128, (qb + 1) * 128)
            m = u_qb == qb
            mp = m & is_pool & valid
            ma = m & is_act & valid
            dv = dev_vals[:, mp]
            ub_q = np.full((128, nu * N_CORES), -np.inf)
            lb_q = np.full((128, nu * N_CORES), -np.inf)
            ub_q[:, mp] = dv + delta - xmin2[mp][None, :]
            lb_q[:, mp] = dv - delta - xmax2[mp][None, :]
            dva = dev_vals[:, ma]
            with np.errstate(divide="ignore"):
                lns = np.log(dva)
            Uq = U_q[qsl].astype(np.float64)[:, None]
            ub_q[:, ma] = Uq + lnT * lns + delta - xmin2[ma][None, :]
            lb_q[:, ma] = (
                Uq + lnT * (lns - np.log(u_w[ma][None, :])) - delta - xmax2[ma][None, :]
            )
            lb_q[:, m & partial] = -np.inf
            ub[qsl] = ub_q
            lb[qsl] = lb_q

        t_q = -np.partition(-lb, K - 1, axis=1)[:, K - 1]
        hits = ub >= t_q[:, None]

        # ---- exact rescan of hit units ------------------------------------
        t_rescan = time.time()
        cand_q = [[] for _ in range(B)]
        max_dev = 0.0
        act_viol = 0.0
        n_rescan_cols = 0
        hit_units = np.nonzero(hits.any(axis=0))[0]
        for u in hit_units:
            vw = u_vw[u]
            if vw == 0:
                continue
            qb = u_qb[u]
            qhit = np.nonzero(hits[:, u])[0]
            qs = qhit[(qhit >= qb * 128) & (qhit < (qb + 1) * 128)]
            if qs.size == 0:
                continue
            pr = qs - qb * 128
            g0 = u_col0[u]
            Xc = Xs[g0 : g0 + vw]
            Sp = tq2m[qs] @ Xc.T  # exact s'
            S = Sp - xn_s[g0 : g0 + vw][None, :]
            if u_kind[u] == "D" and not partial[u]:
                dmax = np.max(np.abs(Sp.max(axis=1) - dev_vals[pr, u]))
                max_dev = max(max_dev, float(dmax))
            elif u_kind[u] == "A":
                ub_sp = (
                    U_q[qs].astype(np.float64)
                    + lnT * np.log(np.maximum(dev_vals[pr, u], 1e-300))
                    + delta
                )
                act_viol = max(act_viol, float(np.max(Sp.max(axis=1) - ub_sp)))
            n_rescan_cols += int(qs.size) * int(vw)
            rr, cc = np.nonzero(S >= t_q[qs, None])
            svals = S[rr, cc]
            gidx = order[g0 + cc]
            qq = qs[rr]
            for r_i in range(rr.size):
                cand_q[qq[r_i]].append((float(svals[r_i]), int(gidx[r_i])))

        ok = (
            max_dev <= delta - 0.05
            and act_viol <= 0.02
            and all(len(lst) >= K for lst in cand_q)
        )
        if MERGE_DEBUG:
            print(
                f"merge attempt={_attempt} delta={delta:.3f} "
                f"max_dev={max_dev:.3f} act_viol={act_viol:.3f} "
                f"hit_units={hit_units.size} rescan_cols={n_rescan_cols} "
                f"ncand_min={min(len(l) for l in cand_q)} "
                f"rescan_t={time.time() - t_rescan:.2f}s ok={ok}",
                file=sys.stderr,
            )
        if ok:
            break
        delta = max(1.5 * delta, 2.0 * max_dev + 0.1)

    post_idx = np.empty((B, K), dtype=np.int64)
    for q in range(B):
        lst = cand_q[q]
        assert len(lst) >= K, f"query {q}: only {len(lst)} candidates"
        lst.sort(key=lambda vc: (-vc[0], vc[1]))
        post_idx[q] = [gi for _, gi in lst[:K]]

    # ---- final loss, mirroring the reference math -------------------------
    T_qm = q_batch @ W
    X_nb = X[post_idx]
    diff = T_qm[:, None, :] - X_nb
    l2 = np.sum(diff * diff, axis=-1, dtype=np.float32)
    post_w = _softmax_f32(-l2 / np.float32(TAU))

    pre_idx_b = pre_indices[q_indices]
    pre_w_b = pre_weights[q_indices]

    p_dense = np.zeros((B, N), np.float32)
    p_dense[rows, pre_idx_b] = pre_w_b
    q_dense = np.zeros((B, N), np.float32)
    q_dense[rows, post_idx] = post_w
    union = (p_dense > 0) | (q_dense > 0)
    p = np.where(union, np.maximum(p_dense, np.float32(EPS)), np.float32(0.0))
    p = p / p.sum(axis=1, keepdims=True)
    q = np.where(union, np.maximum(q_dense, np.float32(EPS)), np.float32(0.0))
    q = q / q.sum(axis=1, keepdims=True)
    logp = np.where(union, np.log(np.maximum(p, np.float32(1e-20))), np.float32(0.0))
    logq = np.where(union, np.log(np.maximum(q, np.float32(1e-20))), np.float32(0.0))
    kl = np.sum(np.where(union, p * (logp - logq), np.float32(0.0)), axis=1)
    loss_knn = np.float32(np.mean(kl))
    loss_reg = np.float32(0.5) * np.float32(np.sum(W * W))
    total_loss = np.float32(BETA) * loss_knn + np.float32(LAMB) * loss_reg
    return (
        np.float32(total_loss),
        np.float32(0.0),
        np.float32(loss_knn),
    )
